# revision 21
# baseline (speedup 1.0000x reference)
"""AttentionBlock (GroupNorm32 + QKV 1x1 conv + 8-head attention + proj + residual)
for Trainium2, data-parallel over batch across 8 NeuronCores.

Contract: kernel(**inputs) takes FULL unsharded numpy inputs (keys as in
setup_inputs()) and returns the FULL [8, 512, 32, 32] float32 output.

Sharding: batch b=8 -> one batch element per core (pure data parallel, no
collectives). Weights are replicated. All heavy matmuls run in bf16 with fp32
PSUM accumulation; GroupNorm statistics, softmax normalization and the
residual add stay in fp32.
"""

import numpy as np
import ml_dtypes

import concourse.bass as bass
import concourse.bacc as bacc
import concourse.tile as tile
from concourse import library_config, mybir
from concourse.bass_utils import run_bass_kernel_spmd

F32 = mybir.dt.float32
BF16 = mybir.dt.bfloat16

# Problem shape (hardcoded per spec)
B, C, H, W = 8, 512, 32, 32
L = H * W                 # 1024
NH = 8                    # attention heads
CH = C // NH              # 64 channels per head
G = 32                    # groupnorm groups
GC = C // G               # 16 channels per group
EPS = 1e-5
P = 128                   # SBUF partitions
CC = C // P               # 4 channel chunks of 128
LT = L // P               # 8 spatial tiles of 128
SCALE2 = 1.0 / np.sqrt(CH)  # combined q*k scale (1/sqrt(sqrt(ch)))^2 = 1/8

_CACHE = {}


def _build_nc():
    """Build the single-core Bass program (SPMD: same program on all 8 cores)."""
    nc = bacc.Bacc(None)

    x_d = nc.declare_dram_parameter("x", [C, L], F32, isOutput=False)
    qkvwT_d = nc.declare_dram_parameter("qkv_wT", [C, 3 * C], BF16, isOutput=False)
    projwT_d = nc.declare_dram_parameter("proj_wT", [C, C], BF16, isOutput=False)
    gnsc_d = nc.declare_dram_parameter("gn_scale", [P, CC], F32, isOutput=False)
    gnbi_d = nc.declare_dram_parameter("gn_bias", [P, CC], F32, isOutput=False)
    qkvbq_d = nc.declare_dram_parameter("qkv_b_q", [P, NH // 2], F32, isOutput=False)
    qkvbk_d = nc.declare_dram_parameter("qkv_b_k", [P, NH // 2], F32, isOutput=False)
    qkvbv_d = nc.declare_dram_parameter("qkv_b_v", [1, C], BF16, isOutput=False)
    projb_d = nc.declare_dram_parameter("proj_b", [P, CC], F32, isOutput=False)
    self_d = nc.declare_dram_parameter("sel_fwd", [P, CC, G], F32, isOutput=False)
    selb_d = nc.declare_dram_parameter("sel_bwd", [G, CC, P], F32, isOutput=False)
    out_d = nc.declare_dram_parameter("out", [C, L], F32, isOutput=True)

    x_v = x_d[:].rearrange("(cc p) l -> cc p l", p=P)
    qw_v = qkvwT_d[:].rearrange("(cc p) o -> cc p o", p=P)
    pw_v = projwT_d[:].rearrange("(cc p) o -> cc p o", p=P)
    out_v = out_d[:].rearrange("(cc p) l -> cc p l", p=P)

    with tile.TileContext(nc) as tc:
        with (
            tc.tile_pool(name="singles", bufs=1) as singles,
            tc.tile_pool(name="small", bufs=4) as small,
            tc.tile_pool(name="exps", bufs=18) as expp,
            tc.tile_pool(name="norm", bufs=3) as normp,
            tc.tile_pool(name="outs", bufs=3) as outp,
            tc.tile_pool(name="psA", bufs=2, space="PSUM") as psA,
            tc.tile_pool(name="psB", bufs=2, space="PSUM") as psB,
            tc.tile_pool(name="dram", bufs=3, space="DRAM") as dpool,
        ):
            # ---------------- input DMAs ----------------
            x_sb = [singles.tile([P, L], F32, tag=f"x{cc}", name=f"x{cc}") for cc in range(CC)]
            for cc in range(CC):
                nc.sync.dma_start(out=x_sb[cc], in_=x_v[cc])
            qw_sb = [singles.tile([P, 3 * C], BF16, tag=f"qw{cc}", name=f"qw{cc}") for cc in range(CC)]
            for cc in range(CC):
                nc.sync.dma_start(out=qw_sb[cc], in_=qw_v[cc])
            pw_sb = [singles.tile([P, C], BF16, tag=f"pw{cc}", name=f"pw{cc}") for cc in range(CC)]
            for cc in range(CC):
                nc.sync.dma_start(out=pw_sb[cc], in_=pw_v[cc])
            gnsc_sb = singles.tile([P, CC], F32, tag="gnsc", name="gnsc")
            nc.sync.dma_start(out=gnsc_sb, in_=gnsc_d[:])
            gnbi_sb = singles.tile([P, CC], F32, tag="gnbi", name="gnbi")
            nc.sync.dma_start(out=gnbi_sb, in_=gnbi_d[:])
            bq_sb = singles.tile([P, NH // 2], F32, tag="bq", name="bq")
            nc.sync.dma_start(out=bq_sb, in_=qkvbq_d[:])
            bk_sb = singles.tile([P, NH // 2], F32, tag="bk", name="bk")
            nc.sync.dma_start(out=bk_sb, in_=qkvbk_d[:])
            bv_sb = singles.tile([1, C], BF16, tag="bv", name="bv")
            nc.sync.dma_start(out=bv_sb, in_=qkvbv_d[:])
            pb_sb = singles.tile([P, CC], F32, tag="pb", name="pb")
            nc.sync.dma_start(out=pb_sb, in_=projb_d[:])
            self_sb = singles.tile([P, CC, G], F32, tag="self", name="self")
            nc.sync.dma_start(out=self_sb, in_=self_d[:])
            selb_sb = singles.tile([G, CC, P], F32, tag="selb", name="selb")
            nc.sync.dma_start(out=selb_sb, in_=selb_d[:])

            ones_sb = singles.tile([1, P], BF16, tag="ones", name="ones")
            nc.vector.memset(ones_sb, 1.0)
            eps_sb = singles.tile([G, 1], F32, tag="eps", name="eps")
            nc.vector.memset(eps_sb, EPS)

            # ---------------- GroupNorm ----------------
            # Per-partition (mean, var) over L, then cross-partition group
            # reduce (16 channels/group) via selector matmuls.
            stats_ps = psA.tile([G, 2], F32, tag="mm", name="mm")
            pe2 = [small.tile([P, 2], F32, tag=f"pe2{cc}", name=f"pe2{cc}") for cc in range(CC)]
            for cc in range(CC):
                st = small.tile([P, 2, 6], F32, tag="bnst", name="bnst")
                for i in range(2):
                    nc.vector.bn_stats(out=st[:, i, :], in_=x_sb[cc][:, i * 512:(i + 1) * 512])
                mv = small.tile([P, 2], F32, tag="bnmv", name="bnmv")
                nc.vector.bn_aggr(out=mv, in_=st)
                nc.vector.tensor_copy(pe2[cc][:, 0:1], mv[:, 0:1])
                # e2 = mu*mu + var
                nc.vector.scalar_tensor_tensor(
                    out=pe2[cc][:, 1:2], in0=mv[:, 0:1], scalar=mv[:, 0:1],
                    in1=mv[:, 1:2], op0=mybir.AluOpType.mult, op1=mybir.AluOpType.add,
                )
            for cc in range(CC):
                nc.tensor.matmul(stats_ps, self_sb[:, cc, :], pe2[cc],
                                 start=(cc == 0), stop=(cc == CC - 1))
            # group mean / var / rstd  (rstd = exp(-0.5*ln(var+eps)))
            msr = small.tile([G, 2], F32, tag="msr", name="msr")
            nc.vector.tensor_scalar_mul(msr[:, 0:1], stats_ps[:, 0:1], 1.0 / GC)
            e2g = small.tile([G, 1], F32, tag="e2g", name="e2g")
            nc.vector.tensor_scalar_mul(e2g, stats_ps[:, 1:2], 1.0 / GC)
            musq = small.tile([G, 1], F32, tag="musq", name="musq")
            nc.vector.tensor_mul(musq, msr[:, 0:1], msr[:, 0:1])
            varg = small.tile([G, 1], F32, tag="varg", name="varg")
            nc.vector.tensor_sub(varg, e2g, musq)
            lnv = small.tile([G, 1], F32, tag="lnv", name="lnv")
            nc.scalar.activation(out=lnv, in_=varg, func=mybir.ActivationFunctionType.Ln,
                                 bias=eps_sb, scale=1.0)
            nc.scalar.activation(out=msr[:, 1:2], in_=lnv,
                                 func=mybir.ActivationFunctionType.Exp, scale=-0.5)

            xn_sb = [singles.tile([P, L], BF16, tag=f"xn{cc}", name=f"xn{cc}") for cc in range(CC)]
            for cc in range(CC):
                mr_ps = psB.tile([P, 2], F32, tag="aa", name="aa")
                nc.tensor.matmul(mr_ps, selb_sb[:, cc, :], msr, start=True, stop=True)
                A_t = small.tile([P, 1], F32, tag="A", name="A")
                nc.vector.tensor_mul(A_t, mr_ps[:, 1:2], gnsc_sb[:, cc:cc + 1])
                mA = small.tile([P, 1], F32, tag="mA", name="mA")
                nc.vector.tensor_mul(mA, mr_ps[:, 0:1], A_t)
                B_t = small.tile([P, 1], F32, tag="B", name="B")
                nc.vector.tensor_sub(B_t, gnbi_sb[:, cc:cc + 1], mA)
                # xn = (x*A + B) in bf16
                nc.vector.tensor_scalar(
                    out=xn_sb[cc], in0=x_sb[cc], scalar1=A_t, scalar2=B_t,
                    op0=mybir.AluOpType.mult, op1=mybir.AluOpType.add,
                )

            # ---------------- QKV projection ----------------
            # q2/k2 per head PAIR: head 2*pr on partitions 0:64, head 2*pr+1 on
            # 64:128 (so per-head score matmuls land on distinct PE row groups
            # and run concurrently).  vT (all heads): [L, (h, ch)]
            q2_sb = [singles.tile([P, L], BF16, tag=f"q2{pr}", name=f"q2{pr}") for pr in range(NH // 2)]
            k2_sb = [singles.tile([P, L], BF16, tag=f"k2{pr}", name=f"k2{pr}") for pr in range(NH // 2)]

            def emit_qk(pr):
                # host-packed qkv_wT column layout: [q-pairs 4x128 | k-pairs 4x128 | v (h,ch) 512]
                for which, dst, bias in ((0, q2_sb[pr], bq_sb), (1, k2_sb[pr], bk_sb)):
                    ps = psA.tile([P, L], F32, tag="mm", name="mm")
                    off = which * C + pr * P
                    for lc in range(2):
                        for cc in range(CC):
                            nc.tensor.matmul(
                                ps[:, lc * 512:(lc + 1) * 512],
                                qw_sb[cc][:, off:off + P],
                                xn_sb[cc][:, lc * 512:(lc + 1) * 512],
                                start=(cc == 0), stop=(cc == CC - 1),
                            )
                    nc.vector.tensor_scalar(
                        out=dst, in0=ps, scalar1=bias[:, pr:pr + 1], scalar2=None,
                        op0=mybir.AluOpType.add,
                    )

            vaug_sb = [singles.tile([P, NH, CH + 1], BF16, tag=f"va{lt}", name=f"va{lt}") for lt in range(LT)]

            def emit_vt(lt):
                v_ps = psB.tile([P, C], F32, tag="aa", name="aa")
                for cc in range(CC):
                    nc.tensor.matmul(
                        v_ps, xn_sb[cc][:, lt * P:(lt + 1) * P], qw_sb[cc][:, 2 * C:],
                        start=(cc == 0), stop=False,
                    )
                nc.tensor.matmul(v_ps, ones_sb, bv_sb, start=False, stop=True)
                nc.vector.tensor_copy(
                    vaug_sb[lt][:, :, 0:CH],
                    v_ps[:].rearrange("p (h ch) -> p h ch", ch=CH),
                )
                nc.vector.memset(vaug_sb[lt][:, :, CH:CH + 1], 1.0)

            # ---------------- attention per head ----------------
            a_sb = [singles.tile([P, L], BF16, tag=f"a{cc}", name=f"a{cc}") for cc in range(CC)]

            def emit_wt_exp(h):
                pr, base = h // 2, (h % 2) * CH
                exps = []
                for st in range(LT):
                    wt_ps = psA.tile([P, L], F32, tag="mm", name="mm")
                    for t in range(2):
                        nc.tensor.matmul(
                            wt_ps[:, t * 512:(t + 1) * 512],
                            k2_sb[pr][base:base + CH, st * P:(st + 1) * P],
                            q2_sb[pr][base:base + CH, t * 512:(t + 1) * 512],
                            start=True, stop=True,
                        )
                    e = expp.tile([P, L], BF16, tag="exp", name="exp")
                    nc.scalar.activation(out=e, in_=wt_ps,
                                         func=mybir.ActivationFunctionType.Exp,
                                         scale=SCALE2)
                    exps.append(e)
                return exps

            def emit_av_norm(h, exps):
                aa_ps = psB.tile([CH + 1, L], F32, tag="aa", name="aa")
                for t in range(2):
                    for st in range(LT):
                        nc.tensor.matmul(
                            aa_ps[:, t * 512:(t + 1) * 512],
                            vaug_sb[st][:, h, :],
                            exps[st][:, t * 512:(t + 1) * 512],
                            start=(st == 0), stop=(st == LT - 1),
                        )
                r_sb = normp.tile([1, L], F32, tag="r", name="r")
                nc.vector.reciprocal(r_sb, aa_ps[CH:CH + 1, :])
                r_dr = dpool.tile([1, L], F32, tag="rdr", name="rdr")
                nc.sync.dma_start(out=r_dr, in_=r_sb)
                rb_sb = normp.tile([CH, L], F32, tag="rb", name="rb")
                nc.sync.dma_start(out=rb_sb, in_=r_dr.to_broadcast([CH, L]))
                po = (h % 2) * CH
                nc.vector.scalar_tensor_tensor(
                    out=a_sb[h // 2][po:po + CH, :], in0=aa_ps[0:CH, :], scalar=1.0,
                    in1=rb_sb, op0=mybir.AluOpType.bypass, op1=mybir.AluOpType.mult,
                )

            # Emission order: qk0 first so ScalarE (exp) starts early, then vT
            # + remaining qk while exp_0 runs, then per-head pipelines.
            emit_qk(0)
            exps0 = emit_wt_exp(0)
            for lt in range(LT):
                emit_vt(lt)
            for pr in range(1, NH // 2):
                emit_qk(pr)
            prev = (0, exps0)
            for h in range(1, NH):
                exps = emit_wt_exp(h)
                emit_av_norm(*prev)
                prev = (h, exps)
            emit_av_norm(*prev)

            # ---------------- output projection + residual ----------------
            for ot in range(CC):
                h_ps = psA.tile([P, L], F32, tag="mm", name="mm")
                for lc in range(2):
                    for cc in range(CC):
                        nc.tensor.matmul(
                            h_ps[:, lc * 512:(lc + 1) * 512],
                            pw_sb[cc][:, ot * P:(ot + 1) * P],
                            a_sb[cc][:, lc * 512:(lc + 1) * 512],
                            start=(cc == 0), stop=(cc == CC - 1),
                        )
                o_sb = outp.tile([P, L], F32, tag="o", name="o")
                nc.vector.scalar_tensor_tensor(
                    out=o_sb, in0=h_ps, scalar=pb_sb[:, ot:ot + 1], in1=x_sb[ot],
                    op0=mybir.AluOpType.add, op1=mybir.AluOpType.add,
                )
                nc.sync.dma_start(out=out_v[ot], in_=o_sb)

    nc.compile()
    return nc


def _host_inputs(x, gn_scale, gn_bias, qkv_w, qkv_b, proj_w, proj_b):
    """Host-side layout prep shared by all cores. Returns (per_core, shared)."""
    bf16 = ml_dtypes.bfloat16
    x = np.asarray(x, dtype=np.float32).reshape(B, C, L)
    qkv_w = np.asarray(qkv_w, dtype=np.float32)
    proj_w = np.asarray(proj_w, dtype=np.float32)
    qkv_b = np.asarray(qkv_b, dtype=np.float32)

    shared = {}
    # pack qkv_wT columns: [q head-pairs 4x128 | k head-pairs 4x128 | v (h,ch)]
    qcols = [c for pr in range(NH // 2)
             for c in list(range(2 * pr * 3 * CH, 2 * pr * 3 * CH + CH))
             + list(range((2 * pr + 1) * 3 * CH, (2 * pr + 1) * 3 * CH + CH))]
    kcols = [c + CH for c in qcols]
    vcols = [h * 3 * CH + 2 * CH + c for h in range(NH) for c in range(CH)]
    perm = qcols + kcols + vcols
    shared["qkv_wT"] = np.ascontiguousarray(qkv_w.T[:, perm]).astype(bf16)  # [C, 3C]
    shared["proj_wT"] = np.ascontiguousarray(proj_w.T).astype(bf16)    # [C, C]
    shared["gn_scale"] = np.ascontiguousarray(
        np.asarray(gn_scale, np.float32).reshape(CC, P).T)             # [P, CC]
    shared["gn_bias"] = np.ascontiguousarray(
        np.asarray(gn_bias, np.float32).reshape(CC, P).T)
    bq = np.stack([np.concatenate([qkv_b[2 * pr * 3 * CH: 2 * pr * 3 * CH + CH],
                                   qkv_b[(2 * pr + 1) * 3 * CH: (2 * pr + 1) * 3 * CH + CH]])
                   for pr in range(NH // 2)], axis=1)
    bk = np.stack([np.concatenate([qkv_b[2 * pr * 3 * CH + CH: 2 * pr * 3 * CH + 2 * CH],
                                   qkv_b[(2 * pr + 1) * 3 * CH + CH: (2 * pr + 1) * 3 * CH + 2 * CH]])
                   for pr in range(NH // 2)], axis=1)
    shared["qkv_b_q"] = np.ascontiguousarray(bq)                       # [P, NH//2]
    shared["qkv_b_k"] = np.ascontiguousarray(bk)                       # [P, NH//2]
    shared["qkv_b_v"] = np.ascontiguousarray(
        qkv_b.reshape(NH, 3, CH)[:, 2, :].reshape(1, C)).astype(bf16)  # [1, C]
    shared["proj_b"] = np.ascontiguousarray(
        np.asarray(proj_b, np.float32).reshape(CC, P).T)
    # selector matrices for groupnorm cross-partition reduction
    sel = np.zeros((P, CC, G), np.float32)
    for cc in range(CC):
        for p in range(P):
            sel[p, cc, cc * (P // GC) + p // GC] = 1.0
    shared["sel_fwd"] = sel
    shared["sel_bwd"] = np.ascontiguousarray(sel.transpose(2, 1, 0))   # [G, CC, P]
    per_core = [{"x": np.ascontiguousarray(x[b])} for b in range(B)]
    return per_core, shared


def _run(inputs, trace=False):
    if "nc" not in _CACHE:
        _CACHE["nc"] = _build_nc()
    nc = _CACHE["nc"]
    per_core, shared = _host_inputs(**inputs)
    in_maps = [{**pc, **shared} for pc in per_core]
    res = run_bass_kernel_spmd(nc, in_maps, core_ids=list(range(B)), trace=trace)
    out = np.stack([res.results[i]["out"] for i in range(B)], axis=0)
    return out.reshape(B, C, H, W).astype(np.float32), res


def kernel(**inputs):
    out, _ = _run(inputs, trace=False)
    return out
